# revision 24
# baseline (speedup 1.0000x reference)
"""Trainium2 Bass kernel for sparse multi-headed attention (bf16 redesign).

Semantics (verified against the reference):
  q = x_q @ Wq.T + bq (per head, dk=32), same for k, v
  for each row s: attend to keys {s-c : c in (5,3,1,0), c <= s}
    score_c[s] = q[s].k[s-c] / sqrt(4)
    p = softmax over valid offsets
    attn[s] = sum_c p_c[s] * v[s-c]
  y = attn @ Wo.T + bo

Sharding: data-parallel over d_stock (8 stocks -> 8 cores). Each core
processes 4 (stock,batch) pairs = 2048 rows, feature-major [256, 2048].

Device design (activations/weights bf16, accumulation fp32 in PSUM):
  - q/k/v projections: PE matmuls, PSUM -> SBUF bf16 via scalar ACT (+bias).
  - scores: DVE products q*k_shift (bf16 2x, pair-group merged ops) -> PE
    selector matmuls pack (pair,head) on PSUM partitions.
  - softmax: ACT exp -> DVE adds + fast reciprocal -> normalized p (bf16).
  - head->feature broadcast of p via selector matmuls (selmk) into PSUM;
    half the tiles evacuated to SBUF by the scalar engine so those
    weighted-V multiplies run in DVE 2x mode, the rest read PSUM at 1x.
  - y = Wo^T @ usum computed feature-major (out features on partitions) so
    bo is a per-partition ACT bias; y stored [256, 2048] bf16, host
    transposes and upcasts.
  - inputs stream over all three DGE rings (sync/scalar hw + gpsimd sw);
    keep-warm matmuls bridge PE gaps so the HAM clock gate stays at 8/8.
"""

import numpy as np

from concourse import bacc, bass, mybir, tile
from concourse.bass_utils import run_bass_kernel_spmd

DS, NB, S, DM, H, DK = 8, 4, 512, 256, 8, 32
CONS = (5, 3, 1, 0)
NCORES = 8
NPAIR = NB  # pairs per core (1 stock x 4 batches)
ROWS = NPAIR * S  # 2048
P = 128
PADC = 8  # zero pad columns in front of k/v for shifted reads
SCALE = 0.5  # 1/sqrt(n_att)

f32 = mybir.dt.float32
bf16 = mybir.dt.bfloat16
Act = mybir.ActivationFunctionType


def _emit(ctx, tc, nc, d, y_dram):
    main = ctx.enter_context(tc.tile_pool(name="main", bufs=1))
    prp = ctx.enter_context(tc.tile_pool(name="prp", bufs=2))
    utp = ctx.enter_context(tc.tile_pool(name="utp", bufs=8))
    # Two persistent PSUM pools (8 banks total), never closed: closing a
    # pool injects multi-us DRAIN barriers into every engine queue.
    # pj: [128, 2, 512] double-bank tiles (projections, keep-warm, bc).
    # sc: [128, 512] single-bank tiles (warmup, scores, y).
    pj_psum = ctx.enter_context(tc.tile_pool(name="pj", bufs=2, space="PSUM"))
    sc_psum = ctx.enter_context(tc.tile_pool(name="sc", bufs=4, space="PSUM"))

    # ---------------- input loads (3 DGE rings) ----------------
    ws = {}
    for name in ("wq", "wk", "wv", "wo"):
        for kch in range(2):
            ws[name, kch] = main.tile([P, DM], bf16, name=f"{name}{kch}")
    bias_t = [main.tile([P, 4], f32, name=f"bias{ch}") for ch in range(2)]
    selkm = [main.tile([P, 224], bf16, name=f"selkm{ch}") for ch in range(2)]
    selmk = [[main.tile([P, P], bf16, name=f"selmk{p}{ch}") for ch in range(2)]
             for p in range(NPAIR)]
    xs = {}
    for name in ("xq", "xk", "xv"):
        for ch in range(2):
            xs[name, ch] = main.tile([P, ROWS], bf16, name=f"{name}{ch}")

    for kch in range(2):
        nc.sync.dma_start(out=ws["wq", kch][:],
                          in_=d["wq"][kch * P:(kch + 1) * P, :])
        nc.scalar.dma_start(out=ws["wk", kch][:],
                            in_=d["wk"][kch * P:(kch + 1) * P, :])
    # q/k inputs in [128, 1024] chunks (pair-group major), 2 hw rings
    for h in range(2):
        for name in ("xq", "xk"):
            for ch in range(2):
                eng = nc.sync if ch == 0 else nc.scalar
                eng.dma_start(
                    out=xs[name, ch][:, h * 1024:(h + 1) * 1024],
                    in_=d[name][ch * P:(ch + 1) * P, h * 1024:(h + 1) * 1024])
    for ch in range(2):
        nc.sync.dma_start(out=bias_t[ch][:], in_=d["bias"][ch * P:(ch + 1) * P, :])
        nc.scalar.dma_start(out=selkm[ch][:], in_=d["selkm"][ch])
    for kch in range(2):
        nc.sync.dma_start(out=ws["wv", kch][:],
                          in_=d["wv"][kch * P:(kch + 1) * P, :])
        nc.scalar.dma_start(out=ws["wo", kch][:],
                            in_=d["wo"][kch * P:(kch + 1) * P, :])
    # v inputs + broadcast selectors on the gpsimd (software DGE) ring
    for ch in range(2):
        for h in range(2):
            nc.gpsimd.dma_start(
                out=xs["xv", ch][:, h * 1024:(h + 1) * 1024],
                in_=d["xv"][ch * P:(ch + 1) * P, h * 1024:(h + 1) * 1024])
    for p in range(NPAIR):
        for ch in range(2):
            nc.gpsimd.dma_start(out=selmk[p][ch][:], in_=d["selmk"][p, ch])

    # ---------------- projected tiles ----------------
    q = [main.tile([P, ROWS], bf16, name=f"q{ch}") for ch in range(2)]
    kv = {}
    for name in ("k", "v"):
        for ch in range(2):
            t = main.tile([P, PADC + ROWS], bf16, name=f"{name}{ch}")
            nc.vector.memset(t[:, 0:PADC], 0.0)
            kv[name, ch] = t

    # ---------------- PE warmup (HAM un-throttle) during DMA ----------
    wtile = main.tile([P, 512], bf16, name="wtile")
    nc.vector.memset(wtile[:], 0.0)
    for i in range(10):
        wps = sc_psum.tile([P, 512], f32, name="wps", tag="sc")
        nc.tensor.matmul(
            wps[:], lhsT=wtile[:, 0:P], rhs=wtile[:], start=True, stop=True)

    BCOL = {"q": 0, "k": 1, "v": 2}

    def project(name, och, pg):
        """One projection unit: out[och-chunk, pair-group pg cols]."""
        xname = "x" + name
        ps = pj_psum.tile([P, 2, S], f32, name="pjt", tag="pj")
        for j in range(2):  # pair within group
            t = 2 * pg + j
            for kch in range(2):
                nc.tensor.matmul(
                    ps[:, j, :],
                    lhsT=ws["w" + name, kch][:, och * P:(och + 1) * P],
                    rhs=xs[xname, kch][:, t * S:(t + 1) * S],
                    start=(kch == 0), stop=(kch == 1))
        bias_ap = bias_t[och][:, BCOL[name]:BCOL[name] + 1]
        if name == "q":
            dst = q[och][:, 2 * pg * S:2 * (pg + 1) * S]
        else:
            dst = kv[name, och][:, PADC + 2 * pg * S:PADC + 2 * (pg + 1) * S]
        nc.scalar.activation(dst, ps[:], Act.Identity, bias=bias_ap)

    # score psum tiles: one [128=(pair,head), 512] bank per offset
    sc_t = [sc_psum.tile([P, 512], f32, name=f"sc{c}", tag="sc")
            for c in range(4)]

    def pr_mul(pg, ch, cp):
        """Products q*k_shift for pair-group pg (pairs 2pg,2pg+1), offset
        pair cp (0: c=5,3 stride 2; 1: c=1,0 stride 1). [128, 2, 2, 512]."""
        c_hi, step = ((5, 2), (1, 1))[cp]
        pr = prp.tile([P, 2, 2, 512], bf16, name="pr", tag="pr")
        q_ap = q[ch][:, 2 * pg * S:]
        q_b = bass.AP(
            tensor=q_ap.tensor, offset=q_ap.offset,
            ap=[q_ap.ap[0], [S, 2], [0, 2], [1, 512]])
        k_ap = kv["k", ch][:, PADC + 2 * pg * S - c_hi:]
        k_v = bass.AP(
            tensor=k_ap.tensor, offset=k_ap.offset,
            ap=[k_ap.ap[0], [S, 2], [step, 2], [1, 512]])
        nc.vector.tensor_mul(pr[:], q_b, k_v)
        return pr

    def sc_mm(cp, pr, pg, ch, start, stop):
        for j in range(2):  # pair within group
            p = 2 * pg + j
            for ci in range(2):  # offset within pair
                nc.tensor.matmul(
                    sc_t[2 * cp + ci][:],
                    lhsT=selkm[ch][:, 96 - 32 * p:224 - 32 * p],
                    rhs=pr[:, j, ci, :],
                    start=(start and j == 0), stop=(stop and j == 1))

    # q/k projections for pairs 0,1 then products/scores; then pairs 2,3
    for pg in range(2):
        for name in ("q", "k"):
            for och in range(2):
                project(name, och, pg)
        for ch in range(2):
            pr = pr_mul(pg, ch, 0)
            sc_mm(0, pr, pg, ch, start=(pg == 0 and ch == 0),
                  stop=(pg == 1 and ch == 1))
    p_sb = main.tile([P, 4 * 512], bf16, name="p_sb")
    for ci in range(2):
        nc.scalar.activation(
            p_sb[:, ci * 512:(ci + 1) * 512], sc_t[ci][:], Act.Exp)
    for pg in range(2):
        for ch in range(2):
            pr = pr_mul(pg, ch, 1)
            sc_mm(1, pr, pg, ch, start=(pg == 0 and ch == 0),
                  stop=(pg == 1 and ch == 1))
    for ci in range(2, 4):
        nc.scalar.activation(
            p_sb[:, ci * 512:(ci + 1) * 512], sc_t[ci][:], Act.Exp)

    # v projections (PE busy through the softmax window), pair-major
    for pg in range(2):
        for och in range(2):
            project("v", och, pg)

    # ---------------- softmax over the 4 offsets ----------------
    # mask: slots with s_loc < c never attend -> p = 0 (covers all pairs)
    nc.vector.memset(p_sb[:, 0:5], 0.0)        # c=5 block
    nc.vector.memset(p_sb[:, 512:515], 0.0)    # c=3 block
    nc.vector.memset(p_sb[:, 1024:1025], 0.0)  # c=1 block
    d1 = main.tile([P, 1024], bf16, name="d1")
    nc.vector.tensor_add(d1[:], p_sb[:, 0:1024], p_sb[:, 1024:2048])
    den = main.tile([P, 512], f32, name="den")
    nc.vector.tensor_add(den[:], d1[:, 0:512], d1[:, 512:1024])
    rcp = main.tile([P, 512], f32, name="rcp")
    nc.vector.reciprocal_approx_fast(rcp[:], den[:])
    rcp_b = main.tile([P, 512], bf16, name="rcp_b")
    nc.vector.tensor_copy(rcp_b[:], rcp[:])
    # normalize in two halves so the first broadcast matmuls start earlier
    p4 = p_sb[:].rearrange("a (c s) -> a c s", c=4)
    rcp_v2 = bass.AP(
        tensor=rcp_b[:].tensor, offset=rcp_b[:].offset,
        ap=[rcp_b[:].ap[0], [0, 2], [1, 512]])
    nc.vector.tensor_mul(p4[:, 0:2, :], p4[:, 0:2, :], rcp_v2)
    nc.vector.tensor_mul(p4[:, 2:4, :], p4[:, 2:4, :], rcp_v2)

    # keep-warm matmuls spaced through the softmax window
    for key_ap in (d1[:, 0:256], rcp_b[:, 0:256]):
        wps = pj_psum.tile([P, 2, S], f32, name="wps", tag="pj")
        nc.tensor.matmul(
            wps[:, 0, 0:256], lhsT=wtile[:, 0:P], rhs=key_ap,
            start=True, stop=True)

    usum = [main.tile([P, ROWS], bf16, name=f"usum{ch}") for ch in range(2)]
    y_sb = [main.tile([P, ROWS], bf16, name=f"ysb{och}") for och in range(2)]

    # ---------------- p broadcast (head -> features) + weighted V ------
    def weighted_v(p, ch):
        """usum[ch] pair-p slice = sum_c bc(p)_c * v_shift_c."""
        uts = []
        for cp in range(2):
            c_hi, step = ((5, 2), (1, 1))[cp]
            bc2 = pj_psum.tile([P, 2, S], f32, name="bc2", tag="pj")
            for ci in range(2):
                nc.tensor.matmul(
                    bc2[:, ci, :],
                    lhsT=selmk[p][ch][:],
                    rhs=p_sb[:, (2 * cp + ci) * 512:(2 * cp + ci + 1) * 512],
                    start=True, stop=True)
            v_ap = kv["v", ch][:, PADC + p * S - c_hi:]
            v_v = bass.AP(
                tensor=v_ap.tensor, offset=v_ap.offset,
                ap=[v_ap.ap[0], [step, 2], [1, 512]])
            ut = utp.tile([P, 2, 512], bf16, name="ut", tag="ut")
            if ch == 0:
                # scalar engine evacuates PSUM so the multiply runs 2x
                ev = utp.tile([P, 2, 512], bf16, name="ev", tag="ut")
                nc.scalar.activation(ev[:], bc2[:], Act.Identity)
                nc.vector.tensor_mul(ut[:], ev[:], v_v)
            else:
                nc.vector.tensor_mul(ut[:], bc2[:], v_v)
            uts.append(ut)
        eng = nc.gpsimd if (p % 2 == 1 and ch == 1) else nc.vector
        a = utp.tile([P, 2, 512], bf16, name="ua", tag="ut")
        eng.tensor_add(a[:], uts[0][:], uts[1][:])
        eng.tensor_add(
            usum[ch][:, p * S:(p + 1) * S], a[:, 0, :], a[:, 1, :])

    def emit_y(half):
        for och in range(2):
            for t in (2 * half, 2 * half + 1):  # 512-col quarters
                yp = sc_psum.tile([P, 512], f32, name="ypt", tag="sc")
                for ich in range(2):
                    nc.tensor.matmul(
                        yp[:],
                        lhsT=ws["wo", ich][:, och * P:(och + 1) * P],
                        rhs=usum[ich][:, t * S:(t + 1) * S],
                        start=(ich == 0), stop=(ich == 1))
                nc.scalar.activation(
                    y_sb[och][:, t * S:(t + 1) * S], yp[:],
                    Act.Identity, bias=bias_t[och][:, 3:4])
            eng = nc.sync if och == 0 else nc.scalar
            eng.dma_start(
                out=y_dram[och * P:(och + 1) * P, half * 1024:(half + 1) * 1024],
                in_=y_sb[och][:, half * 1024:(half + 1) * 1024])

    for p in range(NPAIR):
        for ch in range(2):
            weighted_v(p, ch)
        if p == 1:
            emit_y(0)
    emit_y(1)


def build_nc():
    from contextlib import ExitStack
    nc = bacc.Bacc(trn_type="TRN2", target_bir_lowering=False, debug=False)
    d = {}
    for name in ("xq", "xk", "xv"):
        d[name] = nc.dram_tensor(name, [DM, ROWS], bf16, kind="ExternalInput").ap()
    for name in ("wq", "wk", "wv", "wo"):
        d[name] = nc.dram_tensor(name, [DM, DM], bf16, kind="ExternalInput").ap()
    d["bias"] = nc.dram_tensor("bias", [DM, 4], f32, kind="ExternalInput").ap()
    d["selkm"] = nc.dram_tensor("selkm", [2, P, 224], bf16, kind="ExternalInput").ap()
    d["selmk"] = nc.dram_tensor("selmk", [NPAIR, 2, P, P], bf16, kind="ExternalInput").ap()
    y = nc.dram_tensor("y", [DM, ROWS], bf16, kind="ExternalOutput").ap()
    with tile.TileContext(nc) as tc:
        with ExitStack() as ctx:
            _emit(ctx, tc, nc, d, y)
    nc.compile()
    return nc


def _bf16(a):
    import ml_dtypes
    return np.ascontiguousarray(np.asarray(a, np.float32)).astype(
        ml_dtypes.bfloat16)


def make_shared_inputs(Wq, bq, Wk, bk, Wv, bv, Wo, bo):
    shared = {}
    shared["wq"] = _bf16(np.asarray(Wq, np.float32).T)
    shared["wk"] = _bf16(np.asarray(Wk, np.float32).T)
    shared["wv"] = _bf16(np.asarray(Wv, np.float32).T)
    shared["wo"] = _bf16(np.asarray(Wo, np.float32).T)
    shared["bias"] = np.ascontiguousarray(
        np.stack([bq, bk, bv, bo], axis=1), dtype=np.float32)
    # selkm[ch, d, 96+4ch+d//32] = 0.5: score matmul for pair p uses
    # lhsT = selkm[ch][:, 96-32p:224-32p], landing head sums on psum
    # partition 32p + 4ch + d//32 (pair,head packing).
    selkm = np.zeros((2, P, 224), np.float32)
    for ch in range(2):
        for dd in range(P):
            selkm[ch, dd, 96 + ch * 4 + dd // 32] = SCALE
    shared["selkm"] = _bf16(selkm)
    # selmk[p, ch, 32p+4ch+d//32, d] = 1: broadcast matmul for pair p maps
    # the packed (pair,head) partition back onto feature partitions.
    selmk = np.zeros((NPAIR, 2, P, P), np.float32)
    for p in range(NPAIR):
        for ch in range(2):
            for dd in range(P):
                selmk[p, ch, 32 * p + ch * 4 + dd // 32, dd] = 1.0
    shared["selmk"] = _bf16(selmk)
    return shared


def make_core_inputs(query, key_in, value, core):
    # core i handles stock i: [4, 512, 256] -> feature-major [256, 2048]
    out = {}
    for name, x in (("xq", query), ("xk", key_in), ("xv", value)):
        xi = np.asarray(x[core], dtype=np.float32).reshape(ROWS, DM)
        out[name] = _bf16(xi.T)
    return out


def kernel(query, key_in, value, Wq, bq, Wk, bk, Wv, bv, Wo, bo):
    nc = build_nc()
    shared = make_shared_inputs(Wq, bq, Wk, bk, Wv, bv, Wo, bo)
    in_maps = []
    for core in range(NCORES):
        m = dict(shared)
        m.update(make_core_inputs(query, key_in, value, core))
        in_maps.append(m)
    res = run_bass_kernel_spmd(nc, in_maps, list(range(NCORES))).results
    y = np.stack([
        np.asarray(res[i]["y"], dtype=np.float32).T.reshape(NB, S, DM)
        for i in range(NCORES)])
    return y.astype(np.float32)


# revision 29
# speedup vs baseline: 1.0258x; 1.0258x over previous
"""Trainium2 Bass kernel for sparse multi-headed attention (bf16 redesign).

Semantics (verified against the reference):
  q = x_q @ Wq.T + bq (per head, dk=32), same for k, v
  for each row s: attend to keys {s-c : c in (5,3,1,0), c <= s}
    score_c[s] = q[s].k[s-c] / sqrt(4)
    p = softmax over valid offsets
    attn[s] = sum_c p_c[s] * v[s-c]
  y = attn @ Wo.T + bo

Sharding: data-parallel over d_stock (8 stocks -> 8 cores). Each core
processes 4 (stock,batch) pairs = 2048 rows, feature-major [256, 2048].

Device design (activations/weights bf16, accumulation fp32 in PSUM):
  - q/k/v projections: PE matmuls, PSUM -> SBUF bf16 via scalar ACT (+bias).
  - scores: DVE products q*k_shift (bf16 2x, pair-group merged ops) -> PE
    selector matmuls pack (pair,head) on PSUM partitions.
  - softmax: ACT exp -> DVE adds + fast reciprocal -> normalized p (bf16).
  - head->feature broadcast of p via selector matmuls (selmk) into PSUM;
    half the tiles evacuated to SBUF by the scalar engine so those
    weighted-V multiplies run in DVE 2x mode, the rest read PSUM at 1x.
  - y = Wo^T @ usum computed feature-major (out features on partitions) so
    bo is a per-partition ACT bias; y stored [256, 2048] bf16, host
    transposes and upcasts.
  - inputs stream over all three DGE rings (sync/scalar hw + gpsimd sw);
    keep-warm matmuls bridge PE gaps so the HAM clock gate stays at 8/8.
"""

import numpy as np

from concourse import bacc, bass, mybir, tile
from concourse.bass_utils import run_bass_kernel_spmd

DS, NB, S, DM, H, DK = 8, 4, 512, 256, 8, 32
CONS = (5, 3, 1, 0)
NCORES = 8
NPAIR = NB  # pairs per core (1 stock x 4 batches)
ROWS = NPAIR * S  # 2048
P = 128
PADC = 8  # zero pad columns in front of k/v for shifted reads
SCALE = 0.5  # 1/sqrt(n_att)

f32 = mybir.dt.float32
bf16 = mybir.dt.bfloat16
Act = mybir.ActivationFunctionType




def _raw(ap):
    return bass.AP(tensor=ap.tensor, offset=ap.offset,
                   ap=[list(x) for x in ap.ap])

def _emit(ctx, tc, nc, d, y_dram):
    main = ctx.enter_context(tc.tile_pool(name="main", bufs=1))
    prp = ctx.enter_context(tc.tile_pool(name="prp", bufs=2))
    utp = ctx.enter_context(tc.tile_pool(name="utp", bufs=8))
    # Two persistent PSUM pools (8 banks total), never closed: closing a
    # pool injects multi-us DRAIN barriers into every engine queue.
    # pj: [128, 2, 512] double-bank tiles (projections, keep-warm, bc).
    # sc: [128, 512] single-bank tiles (warmup, scores, y).
    pj_psum = ctx.enter_context(tc.tile_pool(name="pj", bufs=2, space="PSUM"))
    sc_psum = ctx.enter_context(tc.tile_pool(name="sc", bufs=4, space="PSUM"))

    # ---------------- input loads (3 DGE rings) ----------------
    ws = {}
    for name in ("wq", "wk", "wv", "wo"):
        for kch in range(2):
            ws[name, kch] = main.tile([P, DM], bf16, name=f"{name}{kch}")
    bias_t = [main.tile([P, 4], f32, name=f"bias{ch}") for ch in range(2)]
    selkm = [main.tile([P, 224], bf16, name=f"selkm{ch}") for ch in range(2)]
    selmk = [[main.tile([P, P], bf16, name=f"selmk{p}{ch}") for ch in range(2)]
             for p in range(NPAIR)]
    xs = {}
    for name in ("xq", "xk", "xv"):
        for ch in range(2):
            xs[name, ch] = main.tile([P, ROWS], bf16, name=f"{name}{ch}")

    for kch in range(2):
        nc.sync.dma_start(out=ws["wq", kch][:],
                          in_=d["wq"][kch * P:(kch + 1) * P, :])
        nc.scalar.dma_start(out=ws["wk", kch][:],
                            in_=d["wk"][kch * P:(kch + 1) * P, :])
    for ch in range(2):
        nc.sync.dma_start(out=bias_t[ch][:], in_=d["bias"][ch * P:(ch + 1) * P, :])
        nc.scalar.dma_start(out=selkm[ch][:], in_=d["selkm"][ch])
    # q/k inputs in [128, 1024] chunks (pair-group major), 3 rings
    rings = [nc.sync, nc.scalar, nc.gpsimd]
    ri = 0
    for h in range(2):
        for name in ("xq", "xk"):
            for ch in range(2):
                rings[ri % 3].dma_start(
                    out=xs[name, ch][:, h * 1024:(h + 1) * 1024],
                    in_=d[name][ch * P:(ch + 1) * P, h * 1024:(h + 1) * 1024])
                ri += 1
    for kch in range(2):
        nc.sync.dma_start(out=ws["wv", kch][:],
                          in_=d["wv"][kch * P:(kch + 1) * P, :])
        nc.scalar.dma_start(out=ws["wo", kch][:],
                            in_=d["wo"][kch * P:(kch + 1) * P, :])
    # v inputs + broadcast selectors spread over the rings
    ri = 0
    for ch in range(2):
        for h in range(2):
            rings[ri % 3].dma_start(
                out=xs["xv", ch][:, h * 1024:(h + 1) * 1024],
                in_=d["xv"][ch * P:(ch + 1) * P, h * 1024:(h + 1) * 1024])
            ri += 1
    for p in range(NPAIR):
        for ch in range(2):
            rings[ri % 3].dma_start(out=selmk[p][ch][:], in_=d["selmk"][p, ch])
            ri += 1

    # ---------------- projected tiles ----------------
    q = [main.tile([P, ROWS], bf16, name=f"q{ch}") for ch in range(2)]
    kv = {}
    for name in ("k", "v"):
        for ch in range(2):
            t = main.tile([P, PADC + ROWS], bf16, name=f"{name}{ch}")
            nc.vector.memset(t[:, 0:PADC], 0.0)
            kv[name, ch] = t

    # ---------------- PE warmup (HAM un-throttle) during DMA ----------
    wtile = main.tile([P, 512], bf16, name="wtile")
    nc.vector.memset(wtile[:], 0.0)
    for i in range(10):
        wps = sc_psum.tile([P, 512], f32, name="wps", tag="sc")
        nc.tensor.matmul(
            wps[:], lhsT=wtile[:, 0:P], rhs=wtile[:], start=True, stop=True)

    BCOL = {"q": 0, "k": 1, "v": 2}

    def project(name, och, pg):
        """One projection unit: out[och-chunk, pair-group pg cols]."""
        xname = "x" + name
        ps = pj_psum.tile([P, 2, S], f32, name="pjt", tag="pj")
        for j in range(2):  # pair within group
            t = 2 * pg + j
            for kch in range(2):
                nc.tensor.matmul(
                    ps[:, j, :],
                    lhsT=ws["w" + name, kch][:, och * P:(och + 1) * P],
                    rhs=xs[xname, kch][:, t * S:(t + 1) * S],
                    start=(kch == 0), stop=(kch == 1))
        bias_ap = bias_t[och][:, BCOL[name]:BCOL[name] + 1]
        if name == "q":
            dst = q[och][:, 2 * pg * S:2 * (pg + 1) * S]
        else:
            dst = kv[name, och][:, PADC + 2 * pg * S:PADC + 2 * (pg + 1) * S]
        nc.scalar.activation(dst, ps[:], Act.Identity, bias=bias_ap)

    # score psum tiles: one [128=(pair,head), 512] bank per offset
    sc_t = [sc_psum.tile([P, 512], f32, name=f"sc{c}", tag="sc")
            for c in range(4)]

    def pr_mul(pg, ch, cp):
        """Products q*k_shift for pair-group pg (pairs 2pg,2pg+1), offset
        pair cp (0: c=5,3 stride 2; 1: c=1,0 stride 1). [128, 2, 2, 512]."""
        c_hi, step = ((5, 2), (1, 1))[cp]
        pr = prp.tile([P, 2, 2, 512], bf16, name="pr", tag="pr")
        q_ap = q[ch][:, 2 * pg * S:]
        q_b = bass.AP(
            tensor=q_ap.tensor, offset=q_ap.offset,
            ap=[q_ap.ap[0], [S, 2], [0, 2], [1, 512]])
        k_ap = kv["k", ch][:, PADC + 2 * pg * S - c_hi:]
        k_v = bass.AP(
            tensor=k_ap.tensor, offset=k_ap.offset,
            ap=[k_ap.ap[0], [S, 2], [step, 2], [1, 512]])
        nc.vector.tensor_mul(pr[:], q_b, k_v)
        return pr

    def sc_mm(cp, pr, pg, ch, start, stop):
        for j in range(2):  # pair within group
            p = 2 * pg + j
            for ci in range(2):  # offset within pair
                nc.tensor.matmul(
                    sc_t[2 * cp + ci][:],
                    lhsT=selkm[ch][:, 96 - 32 * p:224 - 32 * p],
                    rhs=pr[:, j, ci, :],
                    start=(start and j == 0), stop=(stop and j == 1))

    # q/k projections for pairs 0,1 then products/scores; then pairs 2,3
    for pg in range(2):
        # keep-warm weight loads keyed on the arriving input chunks: PE
        # activity during the DMA wait so the HAM clock gate stays open
        for name in ("xq", "xk"):
            for ch in range(2):
                nc.tensor.load_weights(
                    lhsT=xs[name, ch][:, pg * 1024:pg * 1024 + P])
        for name in ("q", "k"):
            for och in range(2):
                project(name, och, pg)
        for ch in range(2):
            pr = pr_mul(pg, ch, 0)
            sc_mm(0, pr, pg, ch, start=(pg == 0 and ch == 0),
                  stop=(pg == 1 and ch == 1))
    p_sb = main.tile([P, 4 * 512], bf16, name="p_sb")
    for ci in range(2):
        nc.scalar.activation(
            p_sb[:, ci * 512:(ci + 1) * 512], sc_t[ci][:], Act.Exp)
    for pg in range(2):
        for ch in range(2):
            pr = pr_mul(pg, ch, 1)
            sc_mm(1, pr, pg, ch, start=(pg == 0 and ch == 0),
                  stop=(pg == 1 and ch == 1))
    for ci in range(2, 4):
        nc.scalar.activation(
            p_sb[:, ci * 512:(ci + 1) * 512], sc_t[ci][:], Act.Exp)

    # v projections (PE busy through the softmax window), pair-major
    for pg in range(2):
        for och in range(2):
            project("v", och, pg)

    # ---------------- softmax over the 4 offsets ----------------
    # mask: slots with s_loc < c never attend -> p = 0 (covers all pairs)
    nc.vector.memset(p_sb[:, 0:5], 0.0)        # c=5 block
    nc.vector.memset(p_sb[:, 512:515], 0.0)    # c=3 block
    nc.vector.memset(p_sb[:, 1024:1025], 0.0)  # c=1 block
    d1 = main.tile([P, 1024], bf16, name="d1")
    nc.vector.tensor_add(d1[:], p_sb[:, 0:1024], p_sb[:, 1024:2048])
    den = main.tile([P, 512], f32, name="den")
    nc.vector.tensor_add(den[:], d1[:, 0:512], d1[:, 512:1024])
    rcp = main.tile([P, 512], f32, name="rcp")
    nc.vector.reciprocal_approx_fast(rcp[:], den[:])
    rcp_b = main.tile([P, 512], bf16, name="rcp_b")
    nc.vector.tensor_copy(rcp_b[:], rcp[:])
    # normalize in two halves so the first broadcast matmuls start earlier
    p4 = p_sb[:].rearrange("a (c s) -> a c s", c=4)
    rcp_v2 = bass.AP(
        tensor=rcp_b[:].tensor, offset=rcp_b[:].offset,
        ap=[rcp_b[:].ap[0], [0, 2], [1, 512]])
    nc.vector.tensor_mul(p4[:, 0:2, :], p4[:, 0:2, :], rcp_v2)
    nc.vector.tensor_mul(p4[:, 2:4, :], p4[:, 2:4, :], rcp_v2)

    # keep-warm matmuls spaced through the softmax window
    for key_ap in (d1[:, 0:256], rcp_b[:, 0:256]):
        wps = pj_psum.tile([P, 2, S], f32, name="wps", tag="pj")
        nc.tensor.matmul(
            wps[:, 0, 0:256], lhsT=wtile[:, 0:P], rhs=key_ap,
            start=True, stop=True)

    usum = [main.tile([P, ROWS], bf16, name=f"usum{ch}") for ch in range(2)]
    y_sb = [main.tile([P, ROWS], bf16, name=f"ysb{och}") for och in range(2)]

    # ---------------- p broadcast (head -> features) + weighted V ------
    def weighted_v(p, ch):
        """usum[ch] pair-p slice = sum_c bc(p)_c * v_shift_c."""
        uts = []
        for cp in range(2):
            c_hi, step = ((5, 2), (1, 1))[cp]
            bc2 = pj_psum.tile([P, 2, S], f32, name="bc2", tag="pj")
            for ci in range(2):
                nc.tensor.matmul(
                    bc2[:, ci, :],
                    lhsT=selmk[p][ch][:],
                    rhs=p_sb[:, (2 * cp + ci) * 512:(2 * cp + ci + 1) * 512],
                    start=True, stop=True)
            v_ap = kv["v", ch][:, PADC + p * S - c_hi:]
            v_v = bass.AP(
                tensor=v_ap.tensor, offset=v_ap.offset,
                ap=[v_ap.ap[0], [step, 2], [1, 512]])
            ut = utp.tile([P, 2, 512], bf16, name="ut", tag="ut")
            if ch == 0:
                # scalar engine evacuates PSUM so the multiply runs 2x
                ev = utp.tile([P, 2, 512], bf16, name="ev", tag="ut")
                nc.scalar.activation(ev[:], bc2[:], Act.Identity)
                nc.vector.tensor_mul(ut[:], ev[:], v_v)
            else:
                nc.vector.tensor_mul(ut[:], bc2[:], v_v)
            uts.append(ut)
        # gpsimd takes the early pairs' add chains (end pairs stay on the
        # faster DVE so emit_y(1)'s critical path is short)
        eng = nc.gpsimd if (p <= 1 and ch == 1) else nc.vector
        a = utp.tile([P, 2, 512], bf16, name="ua", tag="ut")
        eng.tensor_add(a[:], uts[0][:], uts[1][:])
        eng.tensor_add(
            usum[ch][:, p * S:(p + 1) * S], a[:, 0, :], a[:, 1, :])
        # keep-warm PE activity while the adds run
        nc.tensor.load_weights(lhsT=wtile[:, 0:P])
        nc.tensor.load_weights(lhsT=wtile[:, 0:P])

    def emit_y(half):
        for och in range(2):
            for t in (2 * half, 2 * half + 1):  # 512-col quarters
                yp = sc_psum.tile([P, 512], f32, name="ypt", tag="sc")
                for ich in range(2):
                    nc.tensor.matmul(
                        yp[:],
                        lhsT=ws["wo", ich][:, och * P:(och + 1) * P],
                        rhs=usum[ich][:, t * S:(t + 1) * S],
                        start=(ich == 0), stop=(ich == 1))
                nc.scalar.activation(
                    y_sb[och][:, t * S:(t + 1) * S], yp[:],
                    Act.Identity, bias=bias_t[och][:, 3:4])
                eng = nc.sync if (och + t) % 2 == 0 else nc.scalar
                eng.dma_start(
                    out=y_dram[och * P:(och + 1) * P, t * S:(t + 1) * S],
                    in_=y_sb[och][:, t * S:(t + 1) * S])

    for p in range(NPAIR):
        for ch in range(2):
            weighted_v(p, ch)
        if p == 1:
            emit_y(0)
    emit_y(1)


def build_nc():
    from contextlib import ExitStack
    nc = bacc.Bacc(trn_type="TRN2", target_bir_lowering=False, debug=False)
    d = {}
    for name in ("xq", "xk", "xv"):
        d[name] = nc.dram_tensor(name, [DM, ROWS], bf16, kind="ExternalInput").ap()
    for name in ("wq", "wk", "wv", "wo"):
        d[name] = nc.dram_tensor(name, [DM, DM], bf16, kind="ExternalInput").ap()
    d["bias"] = nc.dram_tensor("bias", [DM, 4], f32, kind="ExternalInput").ap()
    d["selkm"] = nc.dram_tensor("selkm", [2, P, 224], bf16, kind="ExternalInput").ap()
    d["selmk"] = nc.dram_tensor("selmk", [NPAIR, 2, P, P], bf16, kind="ExternalInput").ap()
    y = nc.dram_tensor("y", [DM, ROWS], bf16, kind="ExternalOutput").ap()
    with tile.TileContext(nc) as tc:
        with ExitStack() as ctx:
            _emit(ctx, tc, nc, d, y)
    nc.compile()
    return nc


def _bf16(a):
    import ml_dtypes
    return np.ascontiguousarray(np.asarray(a, np.float32)).astype(
        ml_dtypes.bfloat16)


def make_shared_inputs(Wq, bq, Wk, bk, Wv, bv, Wo, bo):
    shared = {}
    shared["wq"] = _bf16(np.asarray(Wq, np.float32).T)
    shared["wk"] = _bf16(np.asarray(Wk, np.float32).T)
    shared["wv"] = _bf16(np.asarray(Wv, np.float32).T)
    shared["wo"] = _bf16(np.asarray(Wo, np.float32).T)
    shared["bias"] = np.ascontiguousarray(
        np.stack([bq, bk, bv, bo], axis=1), dtype=np.float32)
    # selkm[ch, d, 96+4ch+d//32] = 0.5: score matmul for pair p uses
    # lhsT = selkm[ch][:, 96-32p:224-32p], landing head sums on psum
    # partition 32p + 4ch + d//32 (pair,head packing).
    selkm = np.zeros((2, P, 224), np.float32)
    for ch in range(2):
        for dd in range(P):
            selkm[ch, dd, 96 + ch * 4 + dd // 32] = SCALE
    shared["selkm"] = _bf16(selkm)
    # selmk[p, ch, 32p+4ch+d//32, d] = 1: broadcast matmul for pair p maps
    # the packed (pair,head) partition back onto feature partitions.
    selmk = np.zeros((NPAIR, 2, P, P), np.float32)
    for p in range(NPAIR):
        for ch in range(2):
            for dd in range(P):
                selmk[p, ch, 32 * p + ch * 4 + dd // 32, dd] = 1.0
    shared["selmk"] = _bf16(selmk)
    return shared


def make_core_inputs(query, key_in, value, core):
    # core i handles stock i: [4, 512, 256] -> feature-major [256, 2048]
    out = {}
    for name, x in (("xq", query), ("xk", key_in), ("xv", value)):
        xi = np.asarray(x[core], dtype=np.float32).reshape(ROWS, DM)
        out[name] = _bf16(xi.T)
    return out


def kernel(query, key_in, value, Wq, bq, Wk, bk, Wv, bv, Wo, bo):
    nc = build_nc()
    shared = make_shared_inputs(Wq, bq, Wk, bk, Wv, bv, Wo, bo)
    in_maps = []
    for core in range(NCORES):
        m = dict(shared)
        m.update(make_core_inputs(query, key_in, value, core))
        in_maps.append(m)
    res = run_bass_kernel_spmd(nc, in_maps, list(range(NCORES))).results
    y = np.stack([
        np.asarray(res[i]["y"], dtype=np.float32).T.reshape(NB, S, DM)
        for i in range(NCORES)])
    return y.astype(np.float32)
